# revision 25
# baseline (speedup 1.0000x reference)
"""LSTM classifier kernel for Trainium2, data-parallel over batch on 8 cores.

v4: per-core batch slice of 16, fp16 matmuls, fp32 PSUM accumulation.
  - Gates col-tiled on the PE: i@rows0-15, f@32-47, o@64-79, g@96-111.
  - x-part matmuls for step t+1 are emitted BETWEEN step t's transposes so
    the PE FIFO fills its idle window during the pointwise chain (keeps the
    HAM clock-gate warm and takes the x-part off the critical path).
  - Pointwise split into hidden-column halves (a: cols 0-511, b: 512-1023)
    with separate tiles per half so the two half-chains pipeline across
    engines; all DVE ops pair one SBUF operand (lane base) with one PSUM
    operand (PSUM rebases freely).
  - h -> hT via 4+4 PE transposes per half; hT/tps/h16 split per half so
    step t+1's h-matmuls (ko 0-3) can start as soon as half a is ready.

Self-contained: hardcodes shapes B=128, S=256, I=H=1024, C=1000, 8 cores.
"""

import numpy as np

import concourse.bass as bass
import concourse.mybir as mybir
import concourse.tile as tile
from concourse import bacc
from concourse import bass_utils
from concourse.masks import make_identity

F32 = mybir.dt.float32
F16 = mybir.dt.float16
AF = mybir.ActivationFunctionType
OP = mybir.AluOpType

B, S, I, H, C = 128, 256, 1024, 1024, 1000
NCORES = 8
BC = B // NCORES          # 16 batch rows per core
KO = H // 128             # 8 k-chunks
NG = H                    # per-gate width 1024
NH = 512                  # half width


def build_kernel(n_steps=S):
    nc = bacc.Bacc("TRN2", target_bir_lowering=False, debug=False,
                   enable_asserts=False, num_devices=1)

    xt_d = nc.dram_tensor("xt", [I, n_steps * BC], F16, kind="ExternalInput")
    wxh_d = nc.dram_tensor("wxh", [I, 4, NG], F16, kind="ExternalInput")
    whh_d = nc.dram_tensor("whh", [I, 4, NG], F16, kind="ExternalInput")
    bias_d = nc.dram_tensor("bias", [1, 4, NG], F16, kind="ExternalInput")
    wfc_d = nc.dram_tensor("wfc", [I, 1024], F16, kind="ExternalInput")
    bfc_d = nc.dram_tensor("bfc", [1, 1024], F16, kind="ExternalInput")
    out_d = nc.dram_tensor("out", [BC, C], F32, kind="ExternalOutput")

    with tile.TileContext(nc) as tc:
        with tc.tile_pool(name="const", bufs=1) as cpool, \
             tc.tile_pool(name="xp", bufs=4) as xpool, \
             tc.tile_pool(name="tp", bufs=2) as tpool, \
             tc.tile_pool(name="ps", bufs=2, space="PSUM") as pspool, \
             tc.tile_pool(name="sc", bufs=1, space="PSUM") as scpool, \
             tc.tile_pool(name="tps", bufs=1, space="PSUM") as tpspool:

            wxh = cpool.tile([128, KO, 4, NG], F16)
            whh = cpool.tile([128, KO, 4, NG], F16)
            nc.sync.dma_start(wxh[:, :, :, :],
                              wxh_d.ap().rearrange("(ko p) g n -> p ko g n", p=128))
            nc.sync.dma_start(whh[:, :, :, :],
                              whh_d.ap().rearrange("(ko p) g n -> p ko g n", p=128))
            bias_sb = cpool.tile([1, 4, NG], F16)
            nc.sync.dma_start(bias_sb[:, :, :], bias_d.ap()[:, :, :])
            wfc = cpool.tile([128, KO, 1024], F16)
            nc.sync.dma_start(wfc[:, :, :],
                              wfc_d.ap().rearrange("(ko p) n -> p ko n", p=128))
            bfc_sb = cpool.tile([1, 1024], F16)
            nc.sync.dma_start(bfc_sb[:, :], bfc_d.ap()[:, :])

            ones_sb = cpool.tile([1, BC], F16)
            nc.vector.memset(ones_sb[:, :], 1.0)
            ident64 = cpool.tile([80, BC], F16)
            make_identity(nc, ident64[64:80, :])

            # persistent state, split per hidden half (a: ko 0-3, b: 4-7)
            hT = [[cpool.tile([128, KO // 2, BC], F16, name=f"hT{i}{hf}")
                   for hf in "ab"] for i in range(2)]
            for i in range(2):
                for hf in range(2):
                    nc.vector.memset(hT[i][hf][:, :, :], 0.0)
            # PSUM scratch per half: t2@0:16, c@32:48, tanh_g@96:112
            scr = [scpool.tile([128, NH], F32, name=f"scr{hf}") for hf in range(2)]
            nc.vector.memset(scr[0][32:48, :], 0.0)
            nc.vector.memset(scr[1][32:48, :], 0.0)

            xtr = xt_d.ap().rearrange("(ko p) t -> p ko t", p=128)

            def x_mms(psh, xt_t, half, with_bias, kos=range(KO)):
                """x-part matmuls for one nh half (+bias) into psh[half]."""
                nsl = slice(half * NH, (half + 1) * NH)
                for ko in kos:
                    for g in range(4):
                        nc.tensor.matmul(
                            psh[half][32 * g:32 * g + BC, :],
                            xt_t[:, ko, :], wxh[:, ko, g, nsl],
                            tile_position=(0, 32 * g),
                            start=(ko == 0), stop=False,
                            skip_group_check=True)
                if with_bias:
                    for g in range(4):
                        nc.tensor.matmul(
                            psh[half][32 * g:32 * g + BC, :],
                            ones_sb[:, :], bias_sb[:, g, nsl],
                            tile_position=(0, 32 * g),
                            start=False, stop=False, skip_group_check=True)

            # prologue: x-part + bias for step 0
            xt_prev = xpool.tile([128, KO, BC], F16, tag="xt")
            nc.sync.dma_start(xt_prev[:, :, :], xtr[:, :, 0:BC])
            ps_cur = [pspool.tile([128, NH], F32, tag="gatesa", name="gatesa"),
                      pspool.tile([128, NH], F32, tag="gatesb", name="gatesb")]
            for half in range(2):
                x_mms(ps_cur, xt_prev, half, with_bias=True)

            for t in range(n_steps):
                ps = ps_cur
                hTt = hT[t % 2]
                # h-part: nh outer, ko inner; ko 0-3 reads half a, 4-7 half b
                for nh in range(2):
                    nsl = slice(nh * NH, (nh + 1) * NH)
                    for ko in range(KO):
                        src = hTt[0] if ko < 4 else hTt[1]
                        for g in range(4):
                            nc.tensor.matmul(
                                ps[nh][32 * g:32 * g + BC, :],
                                src[:, ko % 4, :], whh[:, ko, g, nsl],
                                tile_position=(0, 32 * g),
                                start=False, stop=(ko == KO - 1),
                                skip_group_check=True)

                # pointwise, per half
                acts = [tpool.tile([80, NH], F16, tag=f"acts{hf}",
                                   name=f"acts{hf}") for hf in range(2)]
                t1 = [tpool.tile([48, NH], F16, tag=f"t1{hf}",
                                 name=f"t1{hf}") for hf in range(2)]
                tcn = [tpool.tile([80, NH], F16, tag=f"tc{hf}",
                                  name=f"tc{hf}") for hf in range(2)]
                h16 = [tpool.tile([80, NH], F16, tag=f"h16{hf}",
                                  name=f"h16{hf}") for hf in range(2)]
                for hf in range(2):
                    nc.scalar.activation(acts[hf][0:80, :], ps[hf][0:80, :],
                                         AF.Sigmoid)
                    nc.scalar.activation(scr[hf][96:112, :], ps[hf][96:112, :],
                                         AF.Tanh)
                for hf in range(2):
                    nc.vector.tensor_tensor(t1[hf][32:48, :], acts[hf][32:48, :],
                                            scr[hf][32:48, :], OP.mult)
                    nc.vector.tensor_tensor(scr[hf][0:16, :], acts[hf][0:16, :],
                                            scr[hf][96:112, :], OP.mult)
                    nc.vector.tensor_tensor(scr[hf][32:48, :], t1[hf][32:48, :],
                                            scr[hf][0:16, :], OP.add)
                for hf in range(2):
                    nc.scalar.activation(tcn[hf][64:80, :], scr[hf][32:48, :],
                                         AF.Tanh)

                # prefetch + x-part of step t+1 (PE gap filler), interleaved
                # with this step's transposes
                last = t == n_steps - 1
                if not last:
                    xt_nxt = xpool.tile([128, KO, BC], F16, tag="xt")
                    nc.sync.dma_start(xt_nxt[:, :, :],
                                      xtr[:, :, (t + 1) * BC:(t + 2) * BC])
                    ps_nxt = [pspool.tile([128, NH], F32, tag="gatesa",
                                           name="gatesa"),
                              pspool.tile([128, NH], F32, tag="gatesb",
                                           name="gatesb")]
                    # tuned fill: x_h0 plus the first half of x_h1 run
                    # before transp_a (PE busy ~6.0us vs h16_a ready ~6.1,
                    # so transp_a is not delayed, unlike the v10 overshoot)
                    x_mms(ps_nxt, xt_nxt, 0, with_bias=True)
                    x_mms(ps_nxt, xt_nxt, 1, with_bias=False, kos=range(0, 4))

                hTn = hT[(t + 1) % 2]
                # per-half tail kept interleaved so copy_a (which unblocks
                # step t+1's ko0-3 h-matmuls) isn't queued behind h16_b
                nc.vector.tensor_tensor(h16[0][64:80, :], acts[0][64:80, :],
                                        tcn[0][64:80, :], OP.mult)
                tps_a = tpspool.tile([128, KO // 2, BC], F16, tag="tpsa")
                for k in range(4):
                    nc.tensor.transpose(tps_a[:, k, :],
                                        h16[0][64:80, 128 * k:128 * (k + 1)],
                                        ident64[64:80, :])
                nc.vector.tensor_copy(hTn[0][:, :, :], tps_a[:, :, :])

                if not last:
                    x_mms(ps_nxt, xt_nxt, 1, with_bias=True, kos=range(4, KO))

                nc.vector.tensor_tensor(h16[1][64:80, :], acts[1][64:80, :],
                                        tcn[1][64:80, :], OP.mult)
                tps_b = tpspool.tile([128, KO // 2, BC], F16, tag="tpsb")
                for k in range(4):
                    nc.tensor.transpose(tps_b[:, k, :],
                                        h16[1][64:80, 128 * k:128 * (k + 1)],
                                        ident64[64:80, :])
                nc.vector.tensor_copy(hTn[1][:, :, :], tps_b[:, :, :])

                if not last:
                    ps_cur = ps_nxt

            # final FC: out = h_last @ WfcT + bfc
            hTl = hT[n_steps % 2]
            psf = [pspool.tile([128, NH], F32, tag="gatesa", name="gatesa"),
                   pspool.tile([128, NH], F32, tag="gatesb", name="gatesb")]
            for nh in range(2):
                nsl = slice(nh * NH, (nh + 1) * NH)
                for ko in range(KO):
                    src = hTl[0] if ko < 4 else hTl[1]
                    nc.tensor.matmul(psf[nh][0:BC, :], src[:, ko % 4, :],
                                     wfc[:, ko, nsl], start=(ko == 0), stop=False,
                                     skip_group_check=True)
                nc.tensor.matmul(psf[nh][0:BC, :], ones_sb[:, :],
                                 bfc_sb[:, nsl], start=False, stop=True,
                                 skip_group_check=True)
            out_sb = tpool.tile([BC, 1024], F32, tag="osb")
            nc.vector.tensor_copy(out_sb[:, 0:NH], psf[0][0:BC, :])
            nc.vector.tensor_copy(out_sb[:, NH:NG], psf[1][0:BC, :])
            nc.sync.dma_start(out_d.ap()[:, :], out_sb[:, 0:C])

    nc.compile()
    return nc


_NC_CACHE = {}


def _get_nc(n_steps=S):
    if n_steps not in _NC_CACHE:
        _NC_CACHE[n_steps] = build_kernel(n_steps)
    return _NC_CACHE[n_steps]


def _prep_weights(Wxh, bxh, Whh, bhh, Wfc, bfc):
    # gate order in reference along 4H: i, f, g(chat), o -> ours: i, f, o, g
    def arrange(WT):  # WT: [I, 4H]
        blocks = [WT[:, 0:H], WT[:, H:2 * H], WT[:, 3 * H:4 * H], WT[:, 2 * H:3 * H]]
        return np.ascontiguousarray(np.stack(blocks, axis=1)).astype(np.float16)

    wxh = arrange(Wxh.T.astype(np.float32))
    whh = arrange(Whh.T.astype(np.float32))
    b = (bxh + bhh).astype(np.float32)
    bias = np.stack([b[0:H], b[H:2 * H], b[3 * H:4 * H], b[2 * H:3 * H]],
                    axis=0)[None].astype(np.float16)
    wfc = np.zeros((I, 1024), np.float16)
    wfc[:, :C] = Wfc.T.astype(np.float16)
    bfc_p = np.zeros((1, 1024), np.float16)
    bfc_p[0, :C] = bfc.astype(np.float16)
    return wxh, whh, bias, wfc, bfc_p


def kernel_run(x, Wxh, bxh, Whh, bhh, Wfc, bfc, n_steps=S, trace=False,
               tmpdir=None):
    x = np.asarray(x, np.float32)
    wxh, whh, bias, wfc, bfc_p = _prep_weights(
        np.asarray(Wxh), np.asarray(bxh), np.asarray(Whh),
        np.asarray(bhh), np.asarray(Wfc), np.asarray(bfc))
    nc = _get_nc(n_steps)

    in_maps = []
    for core in range(NCORES):
        xc = x[core * BC:(core + 1) * BC, :n_steps]          # [16, S, I]
        xt = np.ascontiguousarray(
            xc.transpose(2, 1, 0).reshape(I, n_steps * BC)).astype(np.float16)
        in_maps.append(dict(xt=xt, wxh=wxh, whh=whh, bias=bias,
                            wfc=wfc, bfc=bfc_p))

    res = bass_utils.run_bass_kernel_spmd(
        nc, in_maps, core_ids=list(range(NCORES)), trace=trace,
        tmpdir=tmpdir)
    out = np.concatenate([r["out"] for r in res.results], axis=0)
    return out.astype(np.float32), res


def kernel(**inputs):
    out, _ = kernel_run(**inputs)
    return out
